# revision 1
# baseline (speedup 1.0000x reference)
"""Multi-head attention (nn_Attention_18528488915211) on 8 Trainium2 NeuronCores.

Sharding: tensor-parallel over heads. 16 heads / 8 cores = 2 heads per core.
Each core computes Q/K/V projections for its 256 columns of Wq/Wk/Wv,
attention for its 2 heads, and a partial output projection with its 256 rows
of Wo. The host sums the 8 partial outputs (the TP all-reduce) and adds bo.

Device kernel layout (fp32 storage, float32r matmuls, rel err ~2e-4):
  - x is fed transposed (xt [dmodel, tokens]) so projections need no
    on-device transpose (fp32 DMA-transpose is unsupported).
  - Q^T,K^T computed as [dhead, t] (weights stationary, Identity+bias
    drain on the otherwise-idle ACT engine); V natural [t, d].
  - Scores computed transposed: S^T[j,i] = K^T(lhsT) . Q^T(rhs); exp on
    ACT with the 1/128 scale folded in; AV keeps V stationary.
  - Softmax row-sums accumulate on the Vector engine (chained adds over
    the exp tiles), freeing the PE from rowsum matmuls; approximate
    reciprocal; per-query normalization of the attention output.
  - Output projection is fused into the attention loop per 512-token
    chunk so its PE work and DMA writes overlap attention's ACT/DVE time.
"""

import ml_dtypes
import numpy as np

P = 128          # partitions
DM = 2048        # dmodel
DH = 128         # dhead
HPC = 2          # heads per core
DC = HPC * DH    # dmodel columns per core (256)
B = 4            # batch
L = 2048         # sequence length
T = B * L        # total tokens (8192)
KS = DM // P     # contraction subtiles (16)
TC = 512         # token chunk (matmul free dim)
NCORES = 8


def _build_nc():
    import concourse.mybir as mybir
    import concourse.tile as tile
    from concourse import bacc

    f32 = mybir.dt.float32
    f32r = mybir.dt.float32r
    bf16 = mybir.dt.bfloat16
    EXP = mybir.ActivationFunctionType.Exp
    IDENT = mybir.ActivationFunctionType.Identity

    nc = bacc.Bacc("TRN2", target_bir_lowering=False, debug=False,
                   num_devices=NCORES)

    xt = nc.dram_tensor("xt", [DM, T], f32r, kind="ExternalInput").ap()
    wq = nc.dram_tensor("wq", [DM, DC], f32r, kind="ExternalInput").ap()
    wk = nc.dram_tensor("wk", [DM, DC], f32r, kind="ExternalInput").ap()
    wv = nc.dram_tensor("wv", [DM, DC], f32r, kind="ExternalInput").ap()
    bq = nc.dram_tensor("bq", [DC], f32, kind="ExternalInput").ap()
    bk = nc.dram_tensor("bk", [DC], f32, kind="ExternalInput").ap()
    bv = nc.dram_tensor("bv", [DC], f32, kind="ExternalInput").ap()
    wo = nc.dram_tensor("wo", [DC, DM], bf16, kind="ExternalInput").ap()
    out = nc.dram_tensor("out", [T, DM], f32, kind="ExternalOutput").ap()

    with tile.TileContext(nc) as tc:
        with (
            tc.tile_pool(name="wpool", bufs=1) as wpool,
            tc.tile_pool(name="xpool", bufs=16) as xpool,
            tc.tile_pool(name="qkv", bufs=1) as qkv,
            tc.tile_pool(name="ptp", bufs=3) as ptp,
            tc.tile_pool(name="misc", bufs=2) as misc,
            tc.tile_pool(name="ps", bufs=3, space="PSUM") as ps,
        ):
            # --- resident weights/constants (wo last: needed latest) ---
            wq_sb = wpool.tile([P, KS, DC], f32r, tag="wq")
            wk_sb = wpool.tile([P, KS, DC], f32r, tag="wk")
            wv_sb = wpool.tile([P, KS, DC], f32r, tag="wv")
            for ks in range(KS):
                for w_sb, w_d in ((wq_sb, wq), (wk_sb, wk), (wv_sb, wv)):
                    nc.sync.dma_start(
                        w_sb[:, ks, :], w_d[ks * P:(ks + 1) * P, :])
            bq_sb = wpool.tile([P, HPC], f32, tag="bq")
            bk_sb = wpool.tile([P, HPC], f32, tag="bk")
            nc.sync.dma_start(bq_sb[:], bq.rearrange("(h d) -> d h", d=P))
            nc.sync.dma_start(bk_sb[:], bk.rearrange("(h d) -> d h", d=P))
            bv_sb = wpool.tile([P, DC], f32, tag="bv")
            nc.sync.dma_start(bv_sb[:], bv[None, :].to_broadcast((P, DC)))
            ones_f32 = wpool.tile([P, P], f32, tag="ones_f32")
            nc.any.memset(ones_f32[:], 1.0)
            ones_sb = wpool.tile([P, P], f32r, tag="ones")
            nc.vector.tensor_scalar_add(ones_sb[:], ones_f32[:], 0.0)
            wo_sb = wpool.tile([P, HPC, DM], bf16, tag="wo")
            nc.sync.dma_start(wo_sb[:], wo.rearrange("(h p) n -> p h n", p=P))

            for b in range(B):
                t0 = b * L
                qt_sb = qkv.tile([P, HPC, L], bf16, tag="qt", name="qt",
                                 bufs=2)
                kt_sb = qkv.tile([P, HPC, L], bf16, tag="kt", name="kt",
                                 bufs=2)
                v_sb = qkv.tile([P, L // P, DC], f32r, tag="v", name="v",
                                bufs=2)
                ot_sb = qkv.tile([P, HPC, L], bf16, tag="ot", name="ot",
                                 bufs=2)

                # ============ Phase A: Q/K/V projections ============
                for tci in range(L // TC):
                    xts = []
                    for ks in range(KS):
                        xt_t = xpool.tile([P, TC], f32r, tag="xt")
                        nc.sync.dma_start(
                            xt_t[:],
                            xt[ks * P:(ks + 1) * P,
                               t0 + tci * TC: t0 + (tci + 1) * TC],
                        )
                        xts.append(xt_t)
                    for w_sb, o_sb, b_sb in ((wq_sb, qt_sb, bq_sb),
                                             (wk_sb, kt_sb, bk_sb)):
                        for h in range(HPC):
                            acc = ps.tile([P, TC], f32, tag="ps", name="qk")
                            for ks in range(KS):
                                nc.tensor.matmul(
                                    acc[:],
                                    w_sb[:, ks, h * DH:(h + 1) * DH],
                                    xts[ks][:],
                                    start=(ks == 0), stop=(ks == KS - 1),
                                )
                            # drain on ACT (idle during projections)
                            nc.scalar.activation(
                                o_sb[:, h, tci * TC:(tci + 1) * TC],
                                acc[:], IDENT, bias=b_sb[:, h:h + 1],
                            )
                    for tb in range(TC // P):
                        acc = ps.tile([P, TC], f32, tag="ps", name="vps")
                        for ks in range(KS):
                            nc.tensor.matmul(
                                acc[:, :DC],
                                xts[ks][:, tb * P:(tb + 1) * P],
                                wv_sb[:, ks, :],
                                start=(ks == 0), stop=(ks == KS - 1),
                            )
                        nc.vector.tensor_add(
                            v_sb[:, tci * (TC // P) + tb, :],
                            acc[:, :DC], bv_sb[:],
                        )

                # ===== Phase B+C: attention with fused output projection =====
                for ic in range(L // TC):
                    for h in range(HPC):
                        hd = slice(h * DH, (h + 1) * DH)
                        q_rhs = qt_sb[:, h, ic * TC:(ic + 1) * TC]
                        ot_ps = ps.tile([P, TC], f32, tag="ot",
                                        name="ot_ps", bufs=2)
                        racc = misc.tile([P, 2 * TC], f32r, tag="racc",
                                         name="racc", bufs=2)
                        pt_prev = None
                        for jp in range(L // P // 2):
                            pt2 = ptp.tile([P, 2 * TC], f32r, tag="pt",
                                           name="pt2")
                            for u in range(2):
                                js = 2 * jp + u
                                st_ps = ps.tile([P, TC], f32, tag="st",
                                                name="st_ps", bufs=3)
                                nc.tensor.matmul(
                                    st_ps[:],
                                    kt_sb[:, h, js * P:(js + 1) * P],
                                    q_rhs, start=True, stop=True,
                                )
                                nc.scalar.activation(
                                    pt2[:, u * TC:(u + 1) * TC], st_ps[:],
                                    EXP, scale=1.0 / DH,
                                )
                                nc.tensor.matmul(
                                    ot_ps[:], v_sb[:, js, hd],
                                    pt2[:, u * TC:(u + 1) * TC],
                                    start=(js == 0), stop=(js == L // P - 1),
                                )
                            # rowsum partials on DVE (frees PE)
                            if jp == 0:
                                pt_prev = pt2
                            elif jp == 1:
                                nc.vector.tensor_add(racc[:], pt_prev[:],
                                                     pt2[:])
                            else:
                                nc.vector.tensor_add(racc[:], racc[:],
                                                     pt2[:])
                        rs_ps = ps.tile([P, TC], f32, tag="st",
                                        name="rs_ps", bufs=3)
                        nc.tensor.matmul(rs_ps[:], ones_sb[:], racc[:, :TC],
                                         start=True, stop=False)
                        nc.tensor.matmul(rs_ps[:], ones_sb[:], racc[:, TC:],
                                         start=False, stop=True)
                        rcp = misc.tile([P, TC], f32, tag="rcp", name="rcp",
                                        bufs=2)
                        nc.vector.reciprocal_approx_fast(rcp[:], rs_ps[:])
                        nc.vector.tensor_mul(
                            ot_sb[:, h, ic * TC:(ic + 1) * TC],
                            ot_ps[:], rcp[:],
                        )
                    # fused partial output projection for this token chunk
                    for tbl in range(TC // P):
                        tb = ic * (TC // P) + tbl
                        for ncl in range(DM // TC):
                            o_ps = ps.tile([P, TC], f32, tag="ps",
                                           name="o_ps")
                            for h in range(HPC):
                                nc.tensor.matmul(
                                    o_ps[:],
                                    ot_sb[:, h, tb * P:(tb + 1) * P],
                                    wo_sb[:, h, ncl * TC:(ncl + 1) * TC],
                                    start=(h == 0), stop=(h == HPC - 1),
                                )
                            o_out = misc.tile([P, TC], f32, tag="oout",
                                              name="oout", bufs=3)
                            nc.any.tensor_copy(o_out[:], o_ps[:])
                            nc.sync.dma_start(
                                out[t0 + tb * P: t0 + (tb + 1) * P,
                                    ncl * TC:(ncl + 1) * TC],
                                o_out[:],
                            )

    nc.compile()
    return nc


_NC_CACHE = None


def kernel(**inputs: np.ndarray) -> np.ndarray:
    from concourse.bass_utils import run_bass_kernel_spmd

    global _NC_CACHE
    x = np.asarray(inputs["x"], dtype=np.float32)
    Wq, bq = np.asarray(inputs["Wq"]), np.asarray(inputs["bq"])
    Wk, bk = np.asarray(inputs["Wk"]), np.asarray(inputs["bk"])
    Wv, bv = np.asarray(inputs["Wv"]), np.asarray(inputs["bv"])
    Wo, bo = np.asarray(inputs["Wo"]), np.asarray(inputs["bo"])

    xt = np.ascontiguousarray(x.reshape(T, DM).T)

    in_maps = []
    for c in range(NCORES):
        sl = slice(c * DC, (c + 1) * DC)
        in_maps.append({
            "xt": xt,
            "wq": np.ascontiguousarray(Wq[:, sl]),
            "wk": np.ascontiguousarray(Wk[:, sl]),
            "wv": np.ascontiguousarray(Wv[:, sl]),
            "bq": np.ascontiguousarray(bq[sl]),
            "bk": np.ascontiguousarray(bk[sl]),
            "bv": np.ascontiguousarray(bv[sl]),
            "wo": np.ascontiguousarray(Wo[sl, :]).astype(ml_dtypes.bfloat16),
        })

    if _NC_CACHE is None:
        _NC_CACHE = _build_nc()
    res = run_bass_kernel_spmd(_NC_CACHE, in_maps, core_ids=list(range(NCORES)))

    acc = res.results[0]["out"].astype(np.float32)
    for c in range(1, NCORES):
        acc = acc + res.results[c]["out"]
    acc = acc + bo[None, :].astype(np.float32)
    return acc.reshape(B, L, DM)



# revision 5
# speedup vs baseline: 1.2543x; 1.2543x over previous
"""Multi-head attention (nn_Attention_18528488915211) on 8 Trainium2 NeuronCores.

Sharding: tensor-parallel over heads. 16 heads / 8 cores = 2 heads per core.
Each core computes Q/K/V projections for its 256 columns of Wq/Wk/Wv,
attention for its 2 heads, and a partial output projection with its 256 rows
of Wo. The host sums the 8 partial outputs (the TP all-reduce) and adds bo.

v2 design (fp16 datapath, PE-bound schedule, measured ~1.0ms -> target ~0.7ms):
  - Everything 16-bit is fp16 (x, weights, q/k/v, probs, partial out):
    rel err ~5e-4 vs 2e-2 budget, and all matmuls run at 1 cycle/row with
    512-element moving operands so LDWEIGHTS (107ns) hides behind each
    213ns matmul.
  - Q^T/K^T/V^T projections are all weights-stationary ([128,128] tiles,
    moving xt [128,512]); V natural layout for the AV matmul is produced
    by the DMA XBAR transpose (SBUF->SBUF, fp16), costing no engine time.
  - Attention per 512-query chunk, heads interleaved, AV software-pipelined
    one key-block behind exp so the PE never waits on the Scalar engine:
    per key-block slot PE does S(h0) S(h1) AV(h0) AV(h1) + 2 output-
    projection filler matmuls (from the previous chunk) = 1278ns vs ACT's
    2 exps = 1140ns.  PSUM: st ring 2 banks + ot_h0/h1 2x2 banks + shared
    proj/out ring 2 banks = 8 banks exactly.
  - Softmax denominators: racc (fp16) accumulates exp tiles on DVE (h0) and
    GPSIMD (h1), reduced over key partitions by a ones-matmul, reciprocal +
    per-query normalize on DVE. Output tiles drain PSUM->SBUF on DVE.
"""

import ml_dtypes
import numpy as np

P = 128          # partitions
DM = 2048        # dmodel
DH = 128         # dhead
HPC = 2          # heads per core
DC = HPC * DH    # dmodel columns per core (256)
B = 4            # batch
L = 2048         # sequence length
T = B * L        # total tokens (8192)
KS = DM // P     # contraction subtiles (16)
TC = 512         # token/query chunk (matmul moving dim)
NCH = L // TC    # chunks per batch (4)
NJ = L // P      # key blocks per batch (16)
NCORES = 8


def _build_nc():
    import concourse.mybir as mybir
    import concourse.tile as tile
    from concourse import bacc

    f32 = mybir.dt.float32
    f16 = mybir.dt.float16
    EXP = mybir.ActivationFunctionType.Exp

    nc = bacc.Bacc("TRN2", target_bir_lowering=False, debug=False,
                   num_devices=NCORES)

    xt = nc.dram_tensor("xt", [DM, T], f16, kind="ExternalInput").ap()
    wq = nc.dram_tensor("wq", [DM, DC], f16, kind="ExternalInput").ap()
    wk = nc.dram_tensor("wk", [DM, DC], f16, kind="ExternalInput").ap()
    wv = nc.dram_tensor("wv", [DM, DC], f16, kind="ExternalInput").ap()
    bq = nc.dram_tensor("bq", [DC], f32, kind="ExternalInput").ap()
    bk = nc.dram_tensor("bk", [DC], f32, kind="ExternalInput").ap()
    bv = nc.dram_tensor("bv", [DC], f32, kind="ExternalInput").ap()
    wo = nc.dram_tensor("wo", [DC, DM], f16, kind="ExternalInput").ap()
    out = nc.dram_tensor("out", [T, DM], f16, kind="ExternalOutput").ap()

    with tile.TileContext(nc) as tc:
        with (
            tc.tile_pool(name="wpool", bufs=1) as wpool,
            tc.tile_pool(name="xpool", bufs=32) as xpool,
            tc.tile_pool(name="qkv", bufs=2) as qkv,
            tc.tile_pool(name="misc", bufs=2) as misc,
            tc.tile_pool(name="psum", bufs=2, space="PSUM") as psum,
        ):
            # --- resident weights/constants ---
            wq_sb = wpool.tile([P, KS, DC], f16, tag="wq")
            wk_sb = wpool.tile([P, KS, DC], f16, tag="wk")
            wv_sb = wpool.tile([P, KS, DC], f16, tag="wv")
            for ks in range(KS):
                for w_sb, w_d in ((wq_sb, wq), (wk_sb, wk), (wv_sb, wv)):
                    nc.sync.dma_start(
                        w_sb[:, ks, :], w_d[ks * P:(ks + 1) * P, :])
            bq_sb = wpool.tile([P, HPC], f32, tag="bq")
            bk_sb = wpool.tile([P, HPC], f32, tag="bk")
            bv_sb = wpool.tile([P, HPC], f32, tag="bv")
            nc.sync.dma_start(bq_sb[:], bq.rearrange("(h d) -> d h", d=P))
            nc.sync.dma_start(bk_sb[:], bk.rearrange("(h d) -> d h", d=P))
            nc.sync.dma_start(bv_sb[:], bv.rearrange("(h d) -> d h", d=P))
            ones_sb = wpool.tile([P, P], f16, tag="ones")
            nc.any.memset(ones_sb[:], 1.0)
            wo_sb = wpool.tile([P, HPC, DM], f16, tag="wo")
            nc.sync.dma_start(wo_sb[:], wo.rearrange("(h p) n -> p h n", p=P))

            # Output-projection work for one finished 512-token chunk,
            # emitted 1 matmul per yield (pumped as PE filler work).
            def o_work_gen(ot_sb, qoff, t0):
                for tb in range(TC // P):
                    tsl = slice(qoff + tb * P, qoff + (tb + 1) * P)
                    for ncl in range(DM // TC):
                        o_ps = psum.tile([P, TC], f32, tag="ps", name="o_ps")
                        nc.tensor.matmul(
                            o_ps[:], ot_sb[:, 0, tsl],
                            wo_sb[:, 0, ncl * TC:(ncl + 1) * TC],
                            start=True, stop=False,
                        )
                        yield
                        nc.tensor.matmul(
                            o_ps[:], ot_sb[:, 1, tsl],
                            wo_sb[:, 1, ncl * TC:(ncl + 1) * TC],
                            start=False, stop=True,
                        )
                        o_sb = misc.tile([P, TC], f16, tag="oout",
                                         name="o_sb", bufs=4)
                        nc.vector.tensor_copy(o_sb[:], o_ps[:])
                        nc.sync.dma_start(
                            out[t0 + tb * P: t0 + (tb + 1) * P,
                                ncl * TC:(ncl + 1) * TC],
                            o_sb[:],
                        )
                        yield

            o_gens = []

            def pump(n):
                while n > 0 and o_gens:
                    try:
                        next(o_gens[0])
                        n -= 1
                    except StopIteration:
                        o_gens.pop(0)

            for b in range(B):
                t0 = b * L
                qt = qkv.tile([P, HPC, L], f16, tag="qt", name="qt")
                kt = qkv.tile([P, HPC, L], f16, tag="kt", name="kt")
                vt = qkv.tile([P, HPC, L], f16, tag="vt", name="vt")
                vn = qkv.tile([P, NJ, HPC, DH], f16, tag="vn", name="vn")
                ot = qkv.tile([P, HPC, L], f16, tag="ot", name="ot")

                # ============ Phase A: Q^T/K^T/V^T projections ============
                for c in range(NCH):
                    cs = slice(c * TC, (c + 1) * TC)
                    xts = []
                    for ks in range(KS):
                        xt_t = xpool.tile([P, TC], f16, tag="xt")
                        nc.sync.dma_start(
                            xt_t[:],
                            xt[ks * P:(ks + 1) * P,
                               t0 + c * TC: t0 + (c + 1) * TC],
                        )
                        xts.append(xt_t)
                    for w_sb, b_sb, dest in ((wq_sb, bq_sb, qt),
                                             (wk_sb, bk_sb, kt),
                                             (wv_sb, bv_sb, vt)):
                        for h in range(HPC):
                            acc = psum.tile([P, TC], f32, tag="ps",
                                            name="proj")
                            for ks in range(KS):
                                nc.tensor.matmul(
                                    acc[:],
                                    w_sb[:, ks, h * DH:(h + 1) * DH],
                                    xts[ks][:],
                                    start=(ks == 0), stop=(ks == KS - 1),
                                )
                            nc.vector.tensor_scalar_add(
                                dest[:, h, cs], acc[:], b_sb[:, h:h + 1])
                    for h in range(HPC):
                        nc.sync.dma_start_transpose(
                            vn[:, c * (TC // P):(c + 1) * (TC // P), h, :],
                            vt[:, h, cs],
                        )

                # ===== Phase B: attention, with fused output projection =====
                for ic in range(NCH):
                    qs = slice(ic * TC, (ic + 1) * TC)
                    ot0 = psum.tile([P, TC], f32, tag="ot0", name="ot0")
                    ot1 = psum.tile([P, TC], f32, tag="ot1", name="ot1")
                    racc = misc.tile([P, HPC, TC], f16, tag="racc",
                                     name="racc")
                    pt_prev = None
                    for js in range(NJ):
                        ksl = slice(js * P, (js + 1) * P)
                        st0 = psum.tile([P, TC], f32, tag="st", name="st")
                        nc.tensor.matmul(st0[:], kt[:, 0, ksl], qt[:, 0, qs],
                                         start=True, stop=True)
                        st1 = psum.tile([P, TC], f32, tag="st", name="st")
                        nc.tensor.matmul(st1[:], kt[:, 1, ksl], qt[:, 1, qs],
                                         start=True, stop=True)
                        pt = misc.tile([P, HPC, TC], f16, tag="pt",
                                       name="pt", bufs=3)
                        nc.scalar.activation(pt[:, 0, :], st0[:], EXP,
                                             scale=1.0 / DH)
                        nc.scalar.activation(pt[:, 1, :], st1[:], EXP,
                                             scale=1.0 / DH)
                        if js == 0:
                            nc.vector.tensor_copy(racc[:, 0, :], pt[:, 0, :])
                            nc.gpsimd.tensor_copy(racc[:, 1, :], pt[:, 1, :])
                        else:
                            nc.vector.tensor_add(racc[:, 0, :], racc[:, 0, :],
                                                 pt[:, 0, :])
                            nc.gpsimd.tensor_add(racc[:, 1, :], racc[:, 1, :],
                                                 pt[:, 1, :])
                        if pt_prev is not None:
                            nc.tensor.matmul(
                                ot0[:], vn[:, js - 1, 0, :], pt_prev[:, 0, :],
                                start=(js == 1), stop=False)
                            nc.tensor.matmul(
                                ot1[:], vn[:, js - 1, 1, :], pt_prev[:, 1, :],
                                start=(js == 1), stop=False)
                        pump(2)
                        pt_prev = pt
                    nc.tensor.matmul(ot0[:], vn[:, NJ - 1, 0, :],
                                     pt_prev[:, 0, :], start=False, stop=True)
                    nc.tensor.matmul(ot1[:], vn[:, NJ - 1, 1, :],
                                     pt_prev[:, 1, :], start=False, stop=True)
                    rs0 = psum.tile([P, TC], f32, tag="ps", name="rs")
                    nc.tensor.matmul(rs0[:], ones_sb[:], racc[:, 0, :],
                                     start=True, stop=True)
                    rs1 = psum.tile([P, TC], f32, tag="ps", name="rs")
                    nc.tensor.matmul(rs1[:], ones_sb[:], racc[:, 1, :],
                                     start=True, stop=True)
                    rcp0 = misc.tile([P, TC], f32, tag="rcp", name="rcp")
                    nc.vector.reciprocal_approx_fast(rcp0[:], rs0[:])
                    nc.vector.tensor_mul(ot[:, 0, qs], ot0[:], rcp0[:])
                    rcp1 = misc.tile([P, TC], f32, tag="rcp", name="rcp")
                    nc.vector.reciprocal_approx_fast(rcp1[:], rs1[:])
                    nc.vector.tensor_mul(ot[:, 1, qs], ot1[:], rcp1[:])
                    o_gens.append(o_work_gen(ot, ic * TC, t0 + ic * TC))

            # tail: drain remaining output-projection work
            pump(1 << 30)

    nc.compile()
    return nc


_NC_CACHE = None


def kernel(**inputs: np.ndarray) -> np.ndarray:
    from concourse.bass_utils import run_bass_kernel_spmd

    global _NC_CACHE
    f16 = np.float16
    x = np.asarray(inputs["x"], dtype=np.float32)
    Wq, bq = np.asarray(inputs["Wq"]), np.asarray(inputs["bq"])
    Wk, bk = np.asarray(inputs["Wk"]), np.asarray(inputs["bk"])
    Wv, bv = np.asarray(inputs["Wv"]), np.asarray(inputs["bv"])
    Wo, bo = np.asarray(inputs["Wo"]), np.asarray(inputs["bo"])

    xt = np.ascontiguousarray(x.reshape(T, DM).T).astype(f16)

    in_maps = []
    for c in range(NCORES):
        sl = slice(c * DC, (c + 1) * DC)
        in_maps.append({
            "xt": xt,
            "wq": np.ascontiguousarray(Wq[:, sl]).astype(f16),
            "wk": np.ascontiguousarray(Wk[:, sl]).astype(f16),
            "wv": np.ascontiguousarray(Wv[:, sl]).astype(f16),
            "bq": np.ascontiguousarray(bq[sl]).astype(np.float32),
            "bk": np.ascontiguousarray(bk[sl]).astype(np.float32),
            "bv": np.ascontiguousarray(bv[sl]).astype(np.float32),
            "wo": np.ascontiguousarray(Wo[sl, :]).astype(f16),
        })

    if _NC_CACHE is None:
        _NC_CACHE = _build_nc()
    res = run_bass_kernel_spmd(_NC_CACHE, in_maps, core_ids=list(range(NCORES)))

    acc = res.results[0]["out"].astype(np.float32)
    for c in range(1, NCORES):
        acc = acc + res.results[c]["out"].astype(np.float32)
    acc = acc + bo[None, :].astype(np.float32)
    return acc.reshape(B, L, DM)


# revision 6
# speedup vs baseline: 1.2672x; 1.0103x over previous
"""Multi-head attention (nn_Attention_18528488915211) on 8 Trainium2 NeuronCores.

Sharding: tensor-parallel over heads. 16 heads / 8 cores = 2 heads per core.
Each core computes Q/K/V projections for its 256 columns of Wq/Wk/Wv,
attention for its 2 heads, and a partial output projection with its 256 rows
of Wo. The host sums the 8 partial outputs (the TP all-reduce) and adds bo.

v2 design (fp16 datapath, PE-bound schedule, measured ~1.0ms -> target ~0.7ms):
  - Everything 16-bit is fp16 (x, weights, q/k/v, probs, partial out):
    rel err ~5e-4 vs 2e-2 budget, and all matmuls run at 1 cycle/row with
    512-element moving operands so LDWEIGHTS (107ns) hides behind each
    213ns matmul.
  - Q^T/K^T/V^T projections are all weights-stationary ([128,128] tiles,
    moving xt [128,512]); V natural layout for the AV matmul is produced
    by the DMA XBAR transpose (SBUF->SBUF, fp16), costing no engine time.
  - Attention per 512-query chunk, heads interleaved, AV software-pipelined
    one key-block behind exp so the PE never waits on the Scalar engine:
    per key-block slot PE does S(h0) S(h1) AV(h0) AV(h1) + 2 output-
    projection filler matmuls (from the previous chunk) = 1278ns vs ACT's
    2 exps = 1140ns.  PSUM: st ring 2 banks + ot_h0/h1 2x2 banks + shared
    proj/out ring 2 banks = 8 banks exactly.
  - Softmax denominators: racc (fp16) accumulates exp tiles on DVE (h0) and
    GPSIMD (h1), reduced over key partitions by a ones-matmul, reciprocal +
    per-query normalize on DVE. Output tiles drain PSUM->SBUF on DVE.
"""

import ml_dtypes
import numpy as np

P = 128          # partitions
DM = 2048        # dmodel
DH = 128         # dhead
HPC = 2          # heads per core
DC = HPC * DH    # dmodel columns per core (256)
B = 4            # batch
L = 2048         # sequence length
T = B * L        # total tokens (8192)
KS = DM // P     # contraction subtiles (16)
TC = 512         # token/query chunk (matmul moving dim)
NCH = L // TC    # chunks per batch (4)
NJ = L // P      # key blocks per batch (16)
NCORES = 8


def _build_nc():
    import concourse.mybir as mybir
    import concourse.tile as tile
    from concourse import bacc

    f32 = mybir.dt.float32
    f16 = mybir.dt.float16
    bf16 = mybir.dt.bfloat16
    EXP = mybir.ActivationFunctionType.Exp

    nc = bacc.Bacc("TRN2", target_bir_lowering=False, debug=False,
                   num_devices=NCORES)

    xt = nc.dram_tensor("xt", [DM, T], f16, kind="ExternalInput").ap()
    wq = nc.dram_tensor("wq", [DM, DC], f16, kind="ExternalInput").ap()
    wk = nc.dram_tensor("wk", [DM, DC], f16, kind="ExternalInput").ap()
    wv = nc.dram_tensor("wv", [DM, DC], f16, kind="ExternalInput").ap()
    bq = nc.dram_tensor("bq", [DC], f32, kind="ExternalInput").ap()
    bk = nc.dram_tensor("bk", [DC], f32, kind="ExternalInput").ap()
    bv = nc.dram_tensor("bv", [DC], f32, kind="ExternalInput").ap()
    wo = nc.dram_tensor("wo", [DC, DM], f16, kind="ExternalInput").ap()
    out = nc.dram_tensor("out", [T, DM], f16, kind="ExternalOutput").ap()

    with tile.TileContext(nc) as tc:
        with (
            tc.tile_pool(name="wpool", bufs=1) as wpool,
            tc.tile_pool(name="xpool", bufs=32) as xpool,
            tc.tile_pool(name="qkv", bufs=2) as qkv,
            tc.tile_pool(name="misc", bufs=2) as misc,
            tc.tile_pool(name="psum", bufs=2, space="PSUM") as psum,
        ):
            # --- resident weights/constants ---
            wq_sb = wpool.tile([P, KS, DC], f16, tag="wq")
            wk_sb = wpool.tile([P, KS, DC], f16, tag="wk")
            wv_sb = wpool.tile([P, KS, DC], f16, tag="wv")
            for ks in range(KS):
                for w_sb, w_d in ((wq_sb, wq), (wk_sb, wk), (wv_sb, wv)):
                    nc.sync.dma_start(
                        w_sb[:, ks, :], w_d[ks * P:(ks + 1) * P, :])
            bq_sb = wpool.tile([P, HPC], f32, tag="bq")
            bk_sb = wpool.tile([P, HPC], f32, tag="bk")
            bv_sb = wpool.tile([P, HPC], f32, tag="bv")
            nc.sync.dma_start(bq_sb[:], bq.rearrange("(h d) -> d h", d=P))
            nc.sync.dma_start(bk_sb[:], bk.rearrange("(h d) -> d h", d=P))
            nc.sync.dma_start(bv_sb[:], bv.rearrange("(h d) -> d h", d=P))
            ones_sb = wpool.tile([P, P], bf16, tag="ones")
            nc.any.memset(ones_sb[:], 1.0)
            wo_sb = wpool.tile([P, HPC, DM], f16, tag="wo")
            nc.sync.dma_start(wo_sb[:], wo.rearrange("(h p) n -> p h n", p=P))

            # Output-projection work for one finished 512-token chunk,
            # emitted 1 matmul per yield (pumped as PE filler work).
            def o_work_gen(ot_sb, qoff, t0):
                for tb in range(TC // P):
                    tsl = slice(qoff + tb * P, qoff + (tb + 1) * P)
                    for ncl in range(DM // TC):
                        o_ps = psum.tile([P, TC], f32, tag="ps", name="o_ps")
                        nc.tensor.matmul(
                            o_ps[:], ot_sb[:, 0, tsl],
                            wo_sb[:, 0, ncl * TC:(ncl + 1) * TC],
                            start=True, stop=False,
                        )
                        yield
                        nc.tensor.matmul(
                            o_ps[:], ot_sb[:, 1, tsl],
                            wo_sb[:, 1, ncl * TC:(ncl + 1) * TC],
                            start=False, stop=True,
                        )
                        o_sb = misc.tile([P, TC], f16, tag="oout",
                                         name="o_sb", bufs=6)
                        nc.vector.tensor_copy(o_sb[:], o_ps[:])
                        nc.sync.dma_start(
                            out[t0 + tb * P: t0 + (tb + 1) * P,
                                ncl * TC:(ncl + 1) * TC],
                            o_sb[:],
                        )
                        yield

            o_gens = []

            def pump(n):
                while n > 0 and o_gens:
                    try:
                        next(o_gens[0])
                        n -= 1
                    except StopIteration:
                        o_gens.pop(0)

            for b in range(B):
                t0 = b * L
                qt = qkv.tile([P, HPC, L], f16, tag="qt", name="qt")
                kt = qkv.tile([P, HPC, L], f16, tag="kt", name="kt")
                vt = qkv.tile([P, HPC, L], bf16, tag="vt", name="vt")
                vn = qkv.tile([P, NJ, HPC, DH], bf16, tag="vn", name="vn")
                ot = qkv.tile([P, HPC, L], f16, tag="ot", name="ot")

                # ============ Phase A: Q^T/K^T/V^T projections ============
                for c in range(NCH):
                    cs = slice(c * TC, (c + 1) * TC)
                    xts = []
                    for ks in range(KS):
                        xt_t = xpool.tile([P, TC], f16, tag="xt")
                        nc.sync.dma_start(
                            xt_t[:],
                            xt[ks * P:(ks + 1) * P,
                               t0 + c * TC: t0 + (c + 1) * TC],
                        )
                        xts.append(xt_t)
                    for w_sb, b_sb, dest in ((wq_sb, bq_sb, qt),
                                             (wk_sb, bk_sb, kt),
                                             (wv_sb, bv_sb, vt)):
                        for h in range(HPC):
                            acc = psum.tile([P, TC], f32, tag="ps",
                                            name="proj")
                            for ks in range(KS):
                                nc.tensor.matmul(
                                    acc[:],
                                    w_sb[:, ks, h * DH:(h + 1) * DH],
                                    xts[ks][:],
                                    start=(ks == 0), stop=(ks == KS - 1),
                                )
                            nc.vector.tensor_scalar_add(
                                dest[:, h, cs], acc[:], b_sb[:, h:h + 1])
                    for h in range(HPC):
                        nc.sync.dma_start_transpose(
                            vn[:, c * (TC // P):(c + 1) * (TC // P), h, :],
                            vt[:, h, cs],
                        )

                # ===== Phase B: attention, with fused output projection =====
                for ic in range(NCH):
                    qs = slice(ic * TC, (ic + 1) * TC)
                    ot0 = psum.tile([P, TC], f32, tag="ot0", name="ot0")
                    ot1 = psum.tile([P, TC], f32, tag="ot1", name="ot1")
                    racc = misc.tile([P, HPC, TC], bf16, tag="racc",
                                     name="racc")
                    pt_prev = None
                    for js in range(NJ):
                        ksl = slice(js * P, (js + 1) * P)
                        st0 = psum.tile([P, TC], f32, tag="st", name="st")
                        nc.tensor.matmul(st0[:], kt[:, 0, ksl], qt[:, 0, qs],
                                         start=True, stop=True)
                        st1 = psum.tile([P, TC], f32, tag="st", name="st")
                        nc.tensor.matmul(st1[:], kt[:, 1, ksl], qt[:, 1, qs],
                                         start=True, stop=True)
                        pt = misc.tile([P, HPC, TC], bf16, tag="pt",
                                       name="pt", bufs=3)
                        nc.scalar.activation(pt[:, 0, :], st0[:], EXP,
                                             scale=1.0 / DH)
                        nc.scalar.activation(pt[:, 1, :], st1[:], EXP,
                                             scale=1.0 / DH)
                        if js == 0:
                            nc.vector.tensor_copy(racc[:, 0, :], pt[:, 0, :])
                            nc.gpsimd.tensor_copy(racc[:, 1, :], pt[:, 1, :])
                        else:
                            nc.vector.tensor_add(racc[:, 0, :], racc[:, 0, :],
                                                 pt[:, 0, :])
                            nc.gpsimd.tensor_add(racc[:, 1, :], racc[:, 1, :],
                                                 pt[:, 1, :])
                        if pt_prev is not None:
                            nc.tensor.matmul(
                                ot0[:], vn[:, js - 1, 0, :], pt_prev[:, 0, :],
                                start=(js == 1), stop=False)
                            nc.tensor.matmul(
                                ot1[:], vn[:, js - 1, 1, :], pt_prev[:, 1, :],
                                start=(js == 1), stop=False)
                        pump(2)
                        pt_prev = pt
                    nc.tensor.matmul(ot0[:], vn[:, NJ - 1, 0, :],
                                     pt_prev[:, 0, :], start=False, stop=True)
                    nc.tensor.matmul(ot1[:], vn[:, NJ - 1, 1, :],
                                     pt_prev[:, 1, :], start=False, stop=True)
                    rs0 = psum.tile([P, TC], f32, tag="ps", name="rs")
                    nc.tensor.matmul(rs0[:], ones_sb[:], racc[:, 0, :],
                                     start=True, stop=True)
                    rs1 = psum.tile([P, TC], f32, tag="ps", name="rs")
                    nc.tensor.matmul(rs1[:], ones_sb[:], racc[:, 1, :],
                                     start=True, stop=True)
                    rcp0 = misc.tile([P, TC], f32, tag="rcp", name="rcp")
                    nc.vector.reciprocal_approx_fast(rcp0[:], rs0[:])
                    nc.vector.tensor_mul(ot[:, 0, qs], ot0[:], rcp0[:])
                    rcp1 = misc.tile([P, TC], f32, tag="rcp", name="rcp")
                    nc.vector.reciprocal_approx_fast(rcp1[:], rs1[:])
                    nc.vector.tensor_mul(ot[:, 1, qs], ot1[:], rcp1[:])
                    o_gens.append(o_work_gen(ot, ic * TC, t0 + ic * TC))

            # tail: drain remaining output-projection work
            pump(1 << 30)

    nc.compile()
    return nc


_NC_CACHE = None


def kernel(**inputs: np.ndarray) -> np.ndarray:
    from concourse.bass_utils import run_bass_kernel_spmd

    global _NC_CACHE
    f16 = np.float16
    x = np.asarray(inputs["x"], dtype=np.float32)
    Wq, bq = np.asarray(inputs["Wq"]), np.asarray(inputs["bq"])
    Wk, bk = np.asarray(inputs["Wk"]), np.asarray(inputs["bk"])
    Wv, bv = np.asarray(inputs["Wv"]), np.asarray(inputs["bv"])
    Wo, bo = np.asarray(inputs["Wo"]), np.asarray(inputs["bo"])

    xt = np.ascontiguousarray(x.reshape(T, DM).T).astype(f16)

    in_maps = []
    for c in range(NCORES):
        sl = slice(c * DC, (c + 1) * DC)
        in_maps.append({
            "xt": xt,
            "wq": np.ascontiguousarray(Wq[:, sl]).astype(f16),
            "wk": np.ascontiguousarray(Wk[:, sl]).astype(f16),
            "wv": np.ascontiguousarray(Wv[:, sl]).astype(f16),
            "bq": np.ascontiguousarray(bq[sl]).astype(np.float32),
            "bk": np.ascontiguousarray(bk[sl]).astype(np.float32),
            "bv": np.ascontiguousarray(bv[sl]).astype(np.float32),
            "wo": np.ascontiguousarray(Wo[sl, :]).astype(f16),
        })

    if _NC_CACHE is None:
        _NC_CACHE = _build_nc()
    res = run_bass_kernel_spmd(_NC_CACHE, in_maps, core_ids=list(range(NCORES)))

    acc = res.results[0]["out"].astype(np.float32)
    for c in range(1, NCORES):
        acc = acc + res.results[c]["out"].astype(np.float32)
    acc = acc + bo[None, :].astype(np.float32)
    return acc.reshape(B, L, DM)


# revision 8
# speedup vs baseline: 1.2706x; 1.0027x over previous
"""Multi-head attention (nn_Attention_18528488915211) on 8 Trainium2 NeuronCores.

Sharding: tensor-parallel over heads. 16 heads / 8 cores = 2 heads per core.
Each core computes Q/K/V projections for its 256 columns of Wq/Wk/Wv,
attention for its 2 heads, and a partial output projection with its 256 rows
of Wo. The host sums the 8 partial outputs (the TP all-reduce) and adds bo.

v3 design (fp16/bf16 datapath, PE-bound software-pipelined schedule):
  - All 16-bit data; every matmul is [128,128] stationary x [128,512] moving
    at 1 cycle/row, so LDWEIGHTS (107ns) hides behind each 213ns matmul.
  - Q^T/K^T/V^T projections weights-stationary; V natural layout for AV is
    produced by the DMA XBAR transpose (SBUF->SBUF), costing no engine time.
  - Attention per 512-query chunk (ic), heads interleaved, AV pipelined one
    key-block behind the Scalar-engine exp; output-projection matmuls of the
    previous chunk fill PE slack from slot 4 on (slots 0-3 cover the chunk's
    rowsum/normalize tail emitted at slot 0).
  - Each chunk's tail (last AV, ones-matmul rowsum, reciprocal, normalize)
    is deferred past the next chunk's first S-pair so the Scalar engine
    never drains; the last chunk of a batch defers its tail into the next
    batch's first projection group.
  - xt tiles for batch b+1 prefetch chunk-by-chunk at each B(b) chunk start
    so output-tile DMAs never queue behind a 4MB burst.
  - PSUM banks: st ring 2 + ot_h0/h1 2x2 + shared proj/out/rowsum ring 2 = 8.
"""

import numpy as np

P = 128          # partitions
DM = 2048        # dmodel
DH = 128         # dhead
HPC = 2          # heads per core
DC = HPC * DH    # dmodel columns per core (256)
B = 4            # batch
L = 2048         # sequence length
T = B * L        # total tokens (8192)
KS = DM // P     # contraction subtiles (16)
TC = 512         # token/query chunk (matmul moving dim)
NCH = L // TC    # chunks per batch (4)
NJ = L // P      # key blocks per batch (16)
NCORES = 8


def _build_nc():
    import concourse.mybir as mybir
    import concourse.tile as tile
    from concourse import bacc

    f32 = mybir.dt.float32
    f16 = mybir.dt.float16
    bf16 = mybir.dt.bfloat16
    EXP = mybir.ActivationFunctionType.Exp

    nc = bacc.Bacc("TRN2", target_bir_lowering=False, debug=False,
                   num_devices=NCORES)

    xt = nc.dram_tensor("xt", [DM, T], f16, kind="ExternalInput").ap()
    wq = nc.dram_tensor("wq", [DM, DC], f16, kind="ExternalInput").ap()
    wk = nc.dram_tensor("wk", [DM, DC], f16, kind="ExternalInput").ap()
    wv = nc.dram_tensor("wv", [DM, DC], f16, kind="ExternalInput").ap()
    bq = nc.dram_tensor("bq", [DC], f32, kind="ExternalInput").ap()
    bk = nc.dram_tensor("bk", [DC], f32, kind="ExternalInput").ap()
    bv = nc.dram_tensor("bv", [DC], f32, kind="ExternalInput").ap()
    wo = nc.dram_tensor("wo", [DC, DM], f16, kind="ExternalInput").ap()
    out = nc.dram_tensor("out", [T, DM], f16, kind="ExternalOutput").ap()

    with tile.TileContext(nc) as tc:
        with (
            tc.tile_pool(name="wpool", bufs=1) as wpool,
            tc.tile_pool(name="xpool", bufs=32) as xpool,
            tc.tile_pool(name="qkv", bufs=2) as qkv,
            tc.tile_pool(name="misc", bufs=2) as misc,
            tc.tile_pool(name="psum", bufs=2, space="PSUM") as psum,
        ):
            xt_cache = {}

            def load_chunk(b, c):
                if (b, c) in xt_cache:
                    return xt_cache.pop((b, c))
                return _claim(b, c)

            def prefetch_chunk(b, c):
                if b < B and (b, c) not in xt_cache:
                    xt_cache[(b, c)] = _claim(b, c)

            def _claim(b, c):
                tiles = []
                for ks in range(KS):
                    xt_t = xpool.tile([P, TC], f16, tag="xt")
                    nc.sync.dma_start(
                        xt_t[:],
                        xt[ks * P:(ks + 1) * P,
                           b * L + c * TC: b * L + (c + 1) * TC],
                    )
                    tiles.append(xt_t)
                return tiles

            # --- resident weights (ordered so the first projection group
            # can start as early as possible) ---
            wq_sb = wpool.tile([P, KS, DC], f16, tag="wq")
            wk_sb = wpool.tile([P, KS, DC], f16, tag="wk")
            wv_sb = wpool.tile([P, KS, DC], f16, tag="wv")
            bq_sb = wpool.tile([P, HPC], f32, tag="bq")
            bk_sb = wpool.tile([P, HPC], f32, tag="bk")
            bv_sb = wpool.tile([P, HPC], f32, tag="bv")
            for ks in range(KS):
                nc.sync.dma_start(wq_sb[:, ks, :], wq[ks * P:(ks + 1) * P, :])
            nc.sync.dma_start(bq_sb[:], bq.rearrange("(h d) -> d h", d=P))
            xt_cache[(0, 0)] = _claim(0, 0)
            for ks in range(KS):
                nc.sync.dma_start(wk_sb[:, ks, :], wk[ks * P:(ks + 1) * P, :])
            nc.sync.dma_start(bk_sb[:], bk.rearrange("(h d) -> d h", d=P))
            xt_cache[(0, 1)] = _claim(0, 1)
            for ks in range(KS):
                nc.sync.dma_start(wv_sb[:, ks, :], wv[ks * P:(ks + 1) * P, :])
            nc.sync.dma_start(bv_sb[:], bv.rearrange("(h d) -> d h", d=P))
            ones_sb = wpool.tile([P, P], bf16, tag="ones")
            nc.any.memset(ones_sb[:], 1.0)
            wo_sb = wpool.tile([P, HPC, DM], f16, tag="wo")
            nc.sync.dma_start(wo_sb[:], wo.rearrange("(h p) n -> p h n", p=P))

            # Output-projection work for one finished 512-token chunk,
            # emitted 1 matmul per yield (pumped as PE filler work).
            def o_work_gen(ot_sb, qoff, t0):
                for tb in range(TC // P):
                    tsl = slice(qoff + tb * P, qoff + (tb + 1) * P)
                    for ncl in range(DM // TC):
                        o_ps = psum.tile([P, TC], f32, tag="ps", name="o_ps")
                        nc.tensor.matmul(
                            o_ps[:], ot_sb[:, 0, tsl],
                            wo_sb[:, 0, ncl * TC:(ncl + 1) * TC],
                            start=True, stop=False,
                        )
                        yield
                        nc.tensor.matmul(
                            o_ps[:], ot_sb[:, 1, tsl],
                            wo_sb[:, 1, ncl * TC:(ncl + 1) * TC],
                            start=False, stop=True,
                        )
                        o_sb = misc.tile([P, TC], f16, tag="oout",
                                         name="o_sb", bufs=6)
                        nc.vector.tensor_copy(o_sb[:], o_ps[:])
                        nc.sync.dma_start(
                            out[t0 + tb * P: t0 + (tb + 1) * P,
                                ncl * TC:(ncl + 1) * TC],
                            o_sb[:],
                        )
                        yield

            o_gens = []

            def pump(n):
                while n > 0 and o_gens:
                    try:
                        next(o_gens[0])
                        n -= 1
                    except StopIteration:
                        o_gens.pop(0)

            # Deferred per-chunk tail: last AV pair, rowsum, recip, normalize.
            pending_tail = [None]

            def run_tail():
                if pending_tail[0] is not None:
                    t, pending_tail[0] = pending_tail[0], None
                    t()

            def make_tail(vn, ot0, ot1, racc, pt_last, ot, qs, qoff, t0):
                def tail():
                    nc.tensor.matmul(ot0[:], vn[:, NJ - 1, 0, :],
                                     pt_last[:, 0, :], start=False, stop=True)
                    nc.tensor.matmul(ot1[:], vn[:, NJ - 1, 1, :],
                                     pt_last[:, 1, :], start=False, stop=True)
                    rs0 = psum.tile([P, TC], f32, tag="ps", name="rs")
                    nc.tensor.matmul(rs0[:], ones_sb[:], racc[:, 0, :],
                                     start=True, stop=True)
                    rs1 = psum.tile([P, TC], f32, tag="ps", name="rs")
                    nc.tensor.matmul(rs1[:], ones_sb[:], racc[:, 1, :],
                                     start=True, stop=True)
                    rcp0 = misc.tile([P, TC], f32, tag="rcp", name="rcp")
                    nc.vector.reciprocal_approx_fast(rcp0[:], rs0[:])
                    nc.vector.tensor_mul(ot[:, 0, qs], ot0[:], rcp0[:])
                    rcp1 = misc.tile([P, TC], f32, tag="rcp", name="rcp")
                    nc.vector.reciprocal_approx_fast(rcp1[:], rs1[:])
                    nc.vector.tensor_mul(ot[:, 1, qs], ot1[:], rcp1[:])
                    o_gens.append(o_work_gen(ot, qoff, t0))
                return tail

            for b in range(B):
                t0 = b * L
                qt = qkv.tile([P, HPC, L], f16, tag="qt", name="qt")
                kt = qkv.tile([P, HPC, L], f16, tag="kt", name="kt")
                vt = qkv.tile([P, HPC, L], bf16, tag="vt", name="vt")
                vn = qkv.tile([P, NJ, HPC, DH], bf16, tag="vn", name="vn")
                ot = qkv.tile([P, HPC, L], f16, tag="ot", name="ot")

                # ============ Phase A: Q^T/K^T/V^T projections ============
                for c in range(NCH):
                    cs = slice(c * TC, (c + 1) * TC)
                    xts = load_chunk(b, c)
                    for w_sb, b_sb, dest in ((wq_sb, bq_sb, qt),
                                             (wk_sb, bk_sb, kt),
                                             (wv_sb, bv_sb, vt)):
                        for h in range(HPC):
                            acc = psum.tile([P, TC], f32, tag="ps",
                                            name="proj")
                            for ks in range(KS):
                                nc.tensor.matmul(
                                    acc[:],
                                    w_sb[:, ks, h * DH:(h + 1) * DH],
                                    xts[ks][:],
                                    start=(ks == 0), stop=(ks == KS - 1),
                                )
                            nc.vector.tensor_scalar_add(
                                dest[:, h, cs], acc[:], b_sb[:, h:h + 1])
                            # previous batch's last-chunk tail rides behind
                            # the first projection group of this batch
                            run_tail()
                    for h in range(HPC):
                        nc.sync.dma_start_transpose(
                            vn[:, c * (TC // P):(c + 1) * (TC // P), h, :],
                            vt[:, h, cs],
                        )

                # ===== Phase B: attention, with fused output projection =====
                for ic in range(NCH):
                    if b + 1 < B:
                        prefetch_chunk(b + 1, ic)
                    qs = slice(ic * TC, (ic + 1) * TC)
                    ot0 = psum.tile([P, TC], f32, tag="ot0", name="ot0")
                    ot1 = psum.tile([P, TC], f32, tag="ot1", name="ot1")
                    racc = misc.tile([P, HPC, TC], bf16, tag="racc",
                                     name="racc")
                    pt_prev = None
                    for js in range(NJ):
                        ksl = slice(js * P, (js + 1) * P)
                        st0 = psum.tile([P, TC], f32, tag="st", name="st")
                        nc.tensor.matmul(st0[:], kt[:, 0, ksl], qt[:, 0, qs],
                                         start=True, stop=True)
                        st1 = psum.tile([P, TC], f32, tag="st", name="st")
                        nc.tensor.matmul(st1[:], kt[:, 1, ksl], qt[:, 1, qs],
                                         start=True, stop=True)
                        if js == 0:
                            run_tail()  # previous chunk's tail
                        pt = misc.tile([P, HPC, TC], bf16, tag="pt",
                                       name="pt", bufs=3)
                        nc.scalar.activation(pt[:, 0, :], st0[:], EXP,
                                             scale=1.0 / DH)
                        nc.scalar.activation(pt[:, 1, :], st1[:], EXP,
                                             scale=1.0 / DH)
                        if js == 1:
                            nc.vector.tensor_add(racc[:, 0, :],
                                                 pt_prev[:, 0, :],
                                                 pt[:, 0, :])
                            nc.gpsimd.tensor_add(racc[:, 1, :],
                                                 pt_prev[:, 1, :],
                                                 pt[:, 1, :])
                        elif js > 1:
                            nc.vector.tensor_add(racc[:, 0, :], racc[:, 0, :],
                                                 pt[:, 0, :])
                            nc.gpsimd.tensor_add(racc[:, 1, :], racc[:, 1, :],
                                                 pt[:, 1, :])
                        if pt_prev is not None:
                            nc.tensor.matmul(
                                ot0[:], vn[:, js - 1, 0, :], pt_prev[:, 0, :],
                                start=(js == 1), stop=False)
                            nc.tensor.matmul(
                                ot1[:], vn[:, js - 1, 1, :], pt_prev[:, 1, :],
                                start=(js == 1), stop=False)
                        if js >= 4:
                            pump(3)
                        pt_prev = pt
                    pending_tail[0] = make_tail(vn, ot0, ot1, racc, pt_prev,
                                                ot, qs, ic * TC, t0 + ic * TC)

            # final flush: last chunk's tail + remaining output projection
            run_tail()
            pump(1 << 30)

    nc.compile()
    return nc


_NC_CACHE = None


def kernel(**inputs: np.ndarray) -> np.ndarray:
    from concourse.bass_utils import run_bass_kernel_spmd

    global _NC_CACHE
    f16 = np.float16
    x = np.asarray(inputs["x"], dtype=np.float32)
    Wq, bq = np.asarray(inputs["Wq"]), np.asarray(inputs["bq"])
    Wk, bk = np.asarray(inputs["Wk"]), np.asarray(inputs["bk"])
    Wv, bv = np.asarray(inputs["Wv"]), np.asarray(inputs["bv"])
    Wo, bo = np.asarray(inputs["Wo"]), np.asarray(inputs["bo"])

    xt = np.ascontiguousarray(x.reshape(T, DM).T).astype(f16)

    in_maps = []
    for c in range(NCORES):
        sl = slice(c * DC, (c + 1) * DC)
        in_maps.append({
            "xt": xt,
            "wq": np.ascontiguousarray(Wq[:, sl]).astype(f16),
            "wk": np.ascontiguousarray(Wk[:, sl]).astype(f16),
            "wv": np.ascontiguousarray(Wv[:, sl]).astype(f16),
            "bq": np.ascontiguousarray(bq[sl]).astype(np.float32),
            "bk": np.ascontiguousarray(bk[sl]).astype(np.float32),
            "bv": np.ascontiguousarray(bv[sl]).astype(np.float32),
            "wo": np.ascontiguousarray(Wo[sl, :]).astype(f16),
        })

    if _NC_CACHE is None:
        _NC_CACHE = _build_nc()
    res = run_bass_kernel_spmd(_NC_CACHE, in_maps, core_ids=list(range(NCORES)))

    acc = res.results[0]["out"].astype(np.float32)
    for c in range(1, NCORES):
        acc = acc + res.results[c]["out"].astype(np.float32)
    acc = acc + bo[None, :].astype(np.float32)
    return acc.reshape(B, L, DM)
